# revision 2
# baseline (speedup 1.0000x reference)
"""Trainium2 Bass kernel for nn_Attend_62534723830373.

Reference computation (note: q is UNUSED by the reference):
    scores = einsum('bhid,bhjd->bhij', k, v) * (1/sqrt(128))
    scores = causal_mask(scores)            # strictly-upper masked
    attn   = softmax(scores, axis=-1)
    out    = einsum('bhij,bhjd->bhid', attn, v)

Shapes: [b=2, h=16, s=2048, d=128] fp32. b*h = 32 head-slices sharded
4-per-core across 8 NeuronCores (data/head parallel, no collectives).

Per-head dataflow on one core (matmul chain in bf16, fp32 accumulate):
  - SWDGE cast-load K, V (fp32 HBM -> bf16 SBUF, natural layout). Head 0
    (blocks 0:8) instead goes HWDGE-fp32 -> DVE cast, because the gpsimd
    engine (the only SWDGE dispatcher) takes ~6us to boot at kernel start.
  - K^T / V^T built by XBAR DMA transposes (dma_start transpose=True,
    SBUF->SBUF, blockwise via 3D out AP) - no PE transposes, no DVE
    PSUM->SBUF copies. identity/lowmask consts are NEFF-baked
    (inline_tensor) and DMA'd, so nothing waits on gpsimd at startup.
  - [V | 1] (130-wide) built with DVE copies + ones memset per head.
  - Per i-chunk (512 wide), j-block pairs share one 1024-wide (2-bank)
    PSUM score tile and ONE exp instruction, emitted with one-pair
    lookahead so the PE always has score matmuls in flight:
      S^T[j, i] = (VT_blk).T @ KT_slice        (PE, contraction d)
      diag pairs: ONE merged matmul adds -2000 strict-lower const to both
        diag blocks via a strided 3D PSUM out AP (identbf @ [mask|mask])
      E = exp(SCALE * S^T)                     (ACT *or* DVE, see below)
      psum_o[i-blk] += E_slice.T @ [V_blk | 1] (PE, contraction j)
    The ones column makes column 128 of each accumulator the softmax
    denominator.
  - exp is load-balanced between the Scalar engine (real ACT exp) and
    the Vector engine. The DVE path computes exp with a Schraudolph
    bit trick: uint16(round(s*A + B)) bit-cast as bf16 equals
    2^(s*SCALE*log2e) within ~2% rms; uint16 saturation at 0 turns
    masked (-2000-biased) scores into bf16 +0.0.
  - out = psum_o[:, 0:128] * (1 / psum_o[:, 128]): reciprocal on DVE,
    then ONE merged scalar_tensor_tensor per po tile (2 i-blocks) with a
    0-stride broadcast of the reciprocals; stored per po tile (2 blocks)
    so the final DMA drains early.

kernel(**inputs) takes FULL unsharded inputs and returns the FULL output.
"""

import numpy as np

B, H, S, D = 2, 16, 2048, 128
N_CORES = 8
HPC = (B * H) // N_CORES  # heads per core = 4
NB = S // 128             # 16 j/i blocks per head
NCH = S // 512            # 4 i-chunks per head
SCALE = 0.08838834764831845
LOG2E = 1.4426950408889634
MASKVAL = -2000.0
EXP_A = float(np.float32(SCALE * 128.0 * LOG2E))
EXP_B = float(np.float32(16256.0 - 7.40))

_CACHED_NC = None


def _build_nc():
    import concourse.bass as bass
    import concourse.mybir as mybir
    import concourse.tile as tile
    from concourse import bacc
    from contextlib import ExitStack
    import ml_dtypes

    f32 = mybir.dt.float32
    bf16 = mybir.dt.bfloat16
    u16 = mybir.dt.uint16
    Exp = mybir.ActivationFunctionType.Exp
    Mult = mybir.AluOpType.mult
    Add = mybir.AluOpType.add

    nc = bacc.Bacc("TRN2", num_devices=N_CORES, debug=False)
    kd = nc.dram_tensor("k", [HPC, S, D], f32, kind="ExternalInput")
    vd = nc.dram_tensor("v", [HPC, S, D], f32, kind="ExternalInput")
    od = nc.dram_tensor("out", [HPC, S, D], f32, kind="ExternalOutput")

    # NEFF-baked constants (loaded to HBM at model load, DMA'd at start)
    np_bf16 = ml_dtypes.bfloat16
    ident_np = np.eye(128, dtype=np_bf16)
    jj, ii = np.meshgrid(np.arange(128), np.arange(128), indexing="ij")
    lm = np.where(jj > ii, np.float32(MASKVAL), np.float32(0.0))
    lm2_np = np.concatenate([lm, lm], axis=1).astype(np_bf16)  # [128, 256]
    ident_dram = nc.inline_tensor(ident_np, name="ident_c")
    lm2_dram = nc.inline_tensor(lm2_np, name="lm2_c")

    # greedy ACT/DVE load balancing (ns cost model incl. seq overhead)
    eng_ns = {"act": 0.0, "dve": 0.0}

    def exp_costs(fd):
        # ns cost models fit from measured traces
        return (fd + 250) / 1.15, (fd + 120) / 0.96 + 45

    def pick(act_cost, dve_cost):
        if eng_ns["act"] + act_cost <= eng_ns["dve"] + dve_cost:
            eng_ns["act"] += act_cost
            return "act"
        eng_ns["dve"] += dve_cost
        return "dve"

    with tile.TileContext(nc) as tc, ExitStack() as ctx:
        const = ctx.enter_context(tc.tile_pool(name="const", bufs=1))
        stagep = ctx.enter_context(tc.tile_pool(name="stage", bufs=1))
        loadp = ctx.enter_context(tc.tile_pool(name="load", bufs=2))
        ktp = ctx.enter_context(tc.tile_pool(name="kt", bufs=2))
        vop = ctx.enter_context(tc.tile_pool(name="vop", bufs=2))
        expp = ctx.enter_context(tc.tile_pool(name="expp", bufs=6))
        outp = ctx.enter_context(tc.tile_pool(name="outp", bufs=2))
        smallp = ctx.enter_context(tc.tile_pool(name="small", bufs=8))
        ps_pool = ctx.enter_context(tc.tile_pool(name="ps", bufs=3, space="PSUM"))
        po_pool = ctx.enter_context(tc.tile_pool(name="po", bufs=2, space="PSUM"))

        # h=0 fp32 staging (HWDGE loads start immediately; gpsimd SWDGE
        # takes ~6us to boot)
        stage_k = stagep.tile([128, 8, 128], f32, tag="stage_k")
        stage_v = stagep.tile([128, 8, 128], f32, tag="stage_v")
        kview0 = kd.ap()[0].rearrange("(n p) d -> p n d", p=128)
        vview0 = vd.ap()[0].rearrange("(n p) d -> p n d", p=128)
        nc.sync.dma_start(stage_k[:, :, :], kview0[:, 0:8, :])
        nc.sync.dma_start(stage_v[:, :, :], vview0[:, 0:8, :])

        identbf = const.tile([128, 128], bf16, tag="identbf")
        nc.sync.dma_start(identbf[:, :], ident_dram.ap())
        lowmask2 = const.tile([128, 256], bf16, tag="lowmask2")
        nc.sync.dma_start(lowmask2[:, :], lm2_dram.ap())
        # warmup exp so ACT's one-time table load happens during startup
        warm = const.tile([128, 1], f32, tag="warm")
        warm2 = const.tile([128, 1], f32, tag="warm2")
        nc.vector.memset(warm[:, :], 0.0)
        nc.scalar.activation(warm2[:, :], warm[:, :], Exp, scale=SCALE)

        for h in range(HPC):
            # ---- loads: fp32 HBM -> bf16 SBUF (SWDGE cast), natural ----
            knat = loadp.tile([128, NB, 128], bf16, tag="knat")
            vnat = loadp.tile([128, NB, 128], bf16, tag="vnat")
            kview = kd.ap()[h].rearrange("(n p) d -> p n d", p=128)
            vview = vd.ap()[h].rearrange("(n p) d -> p n d", p=128)
            vones = vop.tile([128, NB, 130], bf16, tag="vones")
            KT3 = ktp.tile([128, NB, 128], bf16, tag="KT")
            VT3 = ktp.tile([128, NB, 128], bf16, tag="VT")

            if h == 0:
                # DVE cast of the staged fp32, then XBAR transpose
                nc.vector.tensor_copy(knat[:, 0:8, :], stage_k[:, :, :])
                nc.vector.tensor_copy(vnat[:, 0:8, :], stage_v[:, :, :])
                eng_ns["dve"] += 2400
                nc.sync.dma_start(KT3[:, 0:8, :], knat[:, 0:8, :], transpose=True)
                nc.sync.dma_start(VT3[:, 0:8, :], vnat[:, 0:8, :], transpose=True)
                nc.vector.tensor_copy(vones[:, 0:8, 0:128], vnat[:, 0:8, :])
                eng_ns["dve"] += 1250
                nc.gpsimd.dma_start(knat[:, 8:16, :], kview[:, 8:16, :])
                nc.gpsimd.dma_start(vnat[:, 8:16, :], vview[:, 8:16, :])
                nc.sync.dma_start(KT3[:, 8:16, :], knat[:, 8:16, :], transpose=True)
                nc.sync.dma_start(VT3[:, 8:16, :], vnat[:, 8:16, :], transpose=True)
                nc.vector.tensor_copy(vones[:, 8:16, 0:128], vnat[:, 8:16, :])
                eng_ns["dve"] += 1250
            else:
                # first 4 blocks in their own chunk so chunk-0's transposes
                # don't wait on the big load group's queue drain point
                nc.gpsimd.dma_start(knat[:, 0:4, :], kview[:, 0:4, :])
                nc.gpsimd.dma_start(vnat[:, 0:4, :], vview[:, 0:4, :])
                nc.sync.dma_start(KT3[:, 0:4, :], knat[:, 0:4, :], transpose=True)
                nc.sync.dma_start(VT3[:, 0:4, :], vnat[:, 0:4, :], transpose=True)
                nc.gpsimd.dma_start(knat[:, 4:16, :], kview[:, 4:16, :])
                nc.gpsimd.dma_start(vnat[:, 4:16, :], vview[:, 4:16, :])
                nc.sync.dma_start(KT3[:, 4:16, :], knat[:, 4:16, :], transpose=True)
                nc.sync.dma_start(VT3[:, 4:16, :], vnat[:, 4:16, :], transpose=True)
                nc.vector.tensor_copy(vones[:, :, 0:128], vnat[:, :, :])
                eng_ns["dve"] += 2350
            nc.vector.memset(vones[:, :, 128:130], 1.0)
            eng_ns["dve"] += 110
            KT = KT3.rearrange("p n d -> p (n d)")
            VT = VT3.rearrange("p n d -> p (n d)")

            out_sb = outp.tile([128, NB, 128], f32, tag="out_sb")
            oview = od.ap()[h].rearrange("(n p) d -> p n d", p=128)

            # ---- main causal attention loop ----
            for ci in range(NCH):
                i0b = 4 * ci              # first i-block of chunk
                iend = (i0b + 4) * 128
                po = [
                    po_pool.tile([128, 258], f32, tag="po", name=f"po_{h}_{ci}_{u}")
                    for u in range(2)
                ]

                def po_ap(bi):
                    u = bi - i0b
                    return po[u // 2][:, (u % 2) * 129 : (u % 2) * 129 + 129]

                # pairs emitted with one-pair lookahead: pair k+1's score
                # matmuls + exp come before pair k's MM2s, so the PE has
                # work while the first MM2 of a chunk waits for po banks
                pending = None  # (bj_pair_state, ex) awaiting MM2 emission
                pairs = list(range(0, i0b + 4, 2)) + [None]
                for bja in pairs:
                    cur = None
                    if bja is not None:
                        bjb = bja + 1
                        ista = max(i0b, bja) * 128
                        istb_ = max(i0b, bjb) * 128
                        n1a = iend - ista
                        n1b = iend - istb_
                        fd = n1a + n1b
                        ps = ps_pool.tile([128, 1024], f32, tag="ps")
                        # bank of region B: 0 if it fits below col 512
                        same_bank = (n1a + n1b) <= 512
                        diag = bja >= i0b  # diag_a implies diag_b
                        nc.tensor.matmul(
                            ps[:, 0:n1a],
                            VT[:, bja * 128 : (bja + 1) * 128],
                            KT[:, ista:iend],
                            start=True,
                            stop=not diag and not same_bank,
                            skip_group_check=True,
                        )
                        nc.tensor.matmul(
                            ps[:, n1a : n1a + n1b],
                            VT[:, bjb * 128 : (bjb + 1) * 128],
                            KT[:, istb_:iend],
                            start=not same_bank,
                            stop=not diag,
                            skip_group_check=True,
                        )
                        if diag:
                            # one matmul masks BOTH diag blocks: 3D out AP
                            # hits cols [0,128) and [n1a, n1a+128)
                            mview = ps[:, 0 : 2 * n1a].rearrange(
                                "p (two c) -> p two c", two=2
                            )[:, :, 0:128]
                            nc.tensor.matmul(
                                mview,
                                identbf[:, :],
                                lowmask2[:, :],
                                start=False,
                                stop=True,
                                skip_group_check=True,
                            )
                        ex = expp.tile([128, 1024], bf16, tag="ex")
                        ca, cd = exp_costs(fd)
                        if pick(ca, cd) == "act":
                            nc.scalar.activation(
                                ex[:, 0:fd], ps[:, 0:fd], Exp, scale=SCALE
                            )
                        else:
                            nc.vector.tensor_scalar(
                                ex[:, 0:fd].bitcast(u16),
                                ps[:, 0:fd],
                                EXP_A,
                                EXP_B,
                                Mult,
                                Add,
                            )
                        cur = ((bja, ista, 0), (bjb, istb_, n1a), ex)
                    if pending is not None:
                        (pa, pb, pex) = pending
                        for bj, ist, off in (pa, pb):
                            for bi in range(ist // 128, i0b + 4):
                                c0 = off + bi * 128 - ist
                                nc.tensor.matmul(
                                    po_ap(bi),
                                    pex[:, c0 : c0 + 128],
                                    vones[:, bj, 0:129],
                                    start=(bj == 0 and (bi - i0b) % 2 == 0),
                                    stop=(bj == bi and (bi - i0b) % 2 == 1),
                                    skip_group_check=True,
                                )
                    pending = cur
                # epilogue: per po tile (2 i-blocks): strided recip, then
                # ONE merged normalizing multiply (DVE) and the store
                for t in range(2):
                    bi0 = i0b + 2 * t
                    po3 = po[t].rearrange("p (u c) -> p u c", c=129)
                    rc = smallp.tile([128, 2], f32, tag="rc")
                    nc.vector.reciprocal(rc[:, :], po3[:, :, 128])
                    eng_ns["dve"] += 190
                    nc.vector.scalar_tensor_tensor(
                        out_sb[:, bi0 : bi0 + 2, :],
                        po3[:, :, 0:128],
                        1.0,
                        rc[:, :].broadcast_to((128, 2, 128)),
                        mybir.AluOpType.mult,
                        mybir.AluOpType.mult,
                    )
                    eng_ns["dve"] += 440
                    nc.sync.dma_start(
                        oview[:, bi0 : bi0 + 2, :],
                        out_sb[:, bi0 : bi0 + 2, :],
                    )

    nc.finalize()
    return nc


def _get_nc():
    global _CACHED_NC
    if _CACHED_NC is None:
        _CACHED_NC = _build_nc()
    return _CACHED_NC


def run_sharded(k, v, trace=False):
    """k, v: [B*H, S, D] fp32. Returns (out [B*H, S, D], BassKernelResults)."""
    from concourse import bass_utils

    nc = _get_nc()
    in_maps = [
        {
            "k": np.ascontiguousarray(k[c * HPC : (c + 1) * HPC]),
            "v": np.ascontiguousarray(v[c * HPC : (c + 1) * HPC]),
        }
        for c in range(N_CORES)
    ]
    res = bass_utils.run_bass_kernel_spmd(
        nc, in_maps, core_ids=list(range(N_CORES)), trace=trace
    )
    out = np.concatenate([res.results[c]["out"] for c in range(N_CORES)], axis=0)
    return out, res


def kernel(q, k, v):
    k = np.asarray(k, dtype=np.float32).reshape(B * H, S, D)
    v = np.asarray(v, dtype=np.float32).reshape(B * H, S, D)
    out, _ = run_sharded(k, v, trace=False)
    return out.reshape(B, H, S, D)
